# revision 9
# baseline (speedup 1.0000x reference)
"""Bernstein flow density kernel v2 for 8x TRN2 NeuronCores.

Math (per sample n):  density(n) = prod_i s_i,
  s_i = sum_{m=0..15} W''_i-contraction:  s_i = sum_m dk_i[n,m] * x_i^m * (1-x_i)^(15-m)
  dk_i[n,:] = cond_i[n,:] @ W''_i,  W''_i[r,m] = kap[m]*(ca_i[r,m]-ca_i[r,m-1])
  (ca[,-1]=0, ca[,15]=1; folds the psi-difference + binomial scaling into W'').

Per core (8192 samples, p-major: n = p*64 + s), all fp16 on-chip except x/psum:
  - DVE: deg-3 tables, Kron chain k2->k3->cond4 (s-innermost, 2x fp16),
    G5 combine (B4-weighted l-sum), combine mults + tree reductions.
  - PE: batched fp16 transposes of cond4 chunks; per-slot matmuls:
    dims0-4 fold into one cond4^T contraction (partition-of-unity marginals
    baked into wcomb rows); dim5 via G5 = cond4^T @ ca5w.
  - Act: PSUM->SBUF drains (fp16 casts).
  - Pool: x^m / (1-x)^m power tables (phi-hat factors).
"""

import math
import sys

import numpy as np

sys.path.insert(0, "/opt/trn_rl_repo")

import concourse.bacc as bacc  # noqa: E402
import concourse.bass as bass  # noqa: E402
import concourse.tile as tile  # noqa: E402
from concourse import mybir  # noqa: E402
from concourse.bass_utils import run_bass_kernel_spmd  # noqa: E402

N = 65536
DIM = 6
NCORES = 8
NC = N // NCORES          # 8192 samples per core
P = 128
S = NC // P               # 64 samples per partition
G = 8                     # slots per group
NG = S // G               # groups

F32 = mybir.dt.float32
F16 = mybir.dt.float16
MUL = mybir.AluOpType.mult
ADD = mybir.AluOpType.add

_CACHE = {}

def _expand(s8):
    """scale an 8-char per-group config string to NG groups"""
    return "".join(s8[g * 8 // NG] for g in range(NG))

# per-group engine for the ct4 drain: 'a'=Act, 'v'=DVE, 'p'=Pool
CT4_DRAIN = _expand("aaavvaaa")
# per-group engine for the pdk drain
PDK_DRAIN = _expand("aaaaaaaa")
# per-group engine for the cond4 build ('v' or 'p')
C4_ENG = _expand("vvpvppvv")
# per-group engine for the t5 multiply
T5_ENG = _expand("vvvvvvvv")
FAST_START = True
PTR_BUFS = 2
PDK_BUFS = 1
# combine after every 16 slots; trees in 3 batches (32/16/16)
COMBINE_SCHED = {16 // G - 1: [(0, 16)], 32 // G - 1: [(16, 32)],
                 48 // G - 1: [(32, 48)],
                 64 // G - 1: [(48, 56), (56, 64)]}
TREE_SCHED = {32 // G - 1: [(0, 32)], 48 // G - 1: [(32, 48)],
              64 // G - 1: [(48, 56), (56, 64)]}
C4_EARLY = True     # build c4 for late DVE-groups during the early stall
CG_BUFS = 4
DENS_SPLIT = True
CT4P_BUFS = 2
SCR_BUFS = 2
G0_SPLIT = True
U5D5_ENG = _expand("vvvvvvvv")  # per-group engine for the l-sum adds
M1M2_ENG = "vvvvv"      # per combine-batch engine (indexed in order)
TREE_ENG = "vppv"       # per tree-batch engine
# slots >= PHIH_FROM: Pool precomputes phih = px*pq_rev; combine is 1 mult
PHIH_FROM = 32
FASTCAST_ACT = False
G0_PIECES = 2


def _ap(a, off_elems, dims):
    return bass.AP(tensor=a.tensor, offset=a.offset + off_elems, ap=[a.ap[0]] + dims)


def _build_nc():
    nc = bacc.Bacc(target_bir_lowering=False, trn_type="TRN2")

    xr = nc.dram_tensor("xr", [P, S, DIM], F32, kind="ExternalInput")
    wcomb_d = nc.dram_tensor("wcomb", [P, 2, 80], F16, kind="ExternalInput")
    ca5w_d = nc.dram_tensor("ca5w", [P, 2, 64], F16, kind="ExternalInput")
    idh_d = nc.dram_tensor("idh", [P, P], F16, kind="ExternalInput")
    dens_out = nc.dram_tensor("dens", [P, S], F32, kind="ExternalOutput")

    with tile.TileContext(nc) as tc:
        with (
            tc.tile_pool(name="singles", bufs=1) as sg,
            tc.tile_pool(name="cgp", bufs=CG_BUFS) as cgp,
            tc.tile_pool(name="ct4p", bufs=CT4P_BUFS) as ct4p,
            tc.tile_pool(name="scr", bufs=SCR_BUFS) as scr,
            tc.tile_pool(name="ptr", bufs=PTR_BUFS, space="PSUM") as ptrp,
            tc.tile_pool(name="pg5", bufs=2, space="PSUM") as pg5p,
            tc.tile_pool(name="pdk", bufs=PDK_BUFS, space="PSUM") as pdkp,
        ):
            # ---- inputs / consts ----
            xin = sg.tile([P, S, DIM], F32)
            nc.sync.dma_start(out=xin[:, :, :], in_=xr[:, :, :])
            wcomb = sg.tile([P, 2, 80], F16)
            nc.sync.dma_start(out=wcomb[:, :, :], in_=wcomb_d[:, :, :])
            ca5w = sg.tile([P, 2, 64], F16)
            nc.sync.dma_start(out=ca5w[:, :, :], in_=ca5w_d[:, :, :])
            idh = sg.tile([P, P], F16)
            nc.sync.dma_start(out=idh[:, :], in_=idh_d[:, :])

            # ---- casts ----
            # (s,d)-layout fp16 x and 1-x for power chains (on Act: idle early)
            xh6 = sg.tile([P, S, DIM], F16)
            nc.scalar.copy(out=xh6[:, :, :], in_=xin[:, :, :])
            qh6 = sg.tile([P, S, DIM], F16)
            nc.scalar.activation(out=qh6[:, :, :], in_=xin[:, :, :],
                                 func=mybir.ActivationFunctionType.Copy,
                                 scale=-1.0, bias=1.0)
            # d-major fp16 x / 1-x for deg-3 tables (j = 0..4 only)
            NJ = 5
            xh5 = sg.tile([P, NJ, S], F16)
            qh5 = sg.tile([P, NJ, S], F16)
            x2h = sg.tile([P, NJ, S], F16)
            x3h = sg.tile([P, NJ, S], F16)
            q2h = sg.tile([P, NJ, S], F16)
            q3h = sg.tile([P, NJ, S], F16)
            Bbig = sg.tile([P, NJ, 4, S], F16)

            def emit_prep(sl, w, cast_act=False):
                """casts + deg-3 Bernstein tables for slot range [sl, sl+w)."""
                xj = _ap(xin[:, :, :], sl * DIM, [[1, NJ], [DIM, w]])
                d5 = [[S, NJ], [1, w]]
                db = [[4 * S, NJ], [1, w]]
                xs = _ap(xh5[:, :, :], sl, d5)
                qs = _ap(qh5[:, :, :], sl, d5)
                x2s = _ap(x2h[:, :, :], sl, d5)
                q2s = _ap(q2h[:, :, :], sl, d5)
                if cast_act:
                    nc.scalar.copy(out=xs, in_=xj)
                    nc.scalar.activation(out=qs, in_=xj,
                                         func=mybir.ActivationFunctionType.Copy,
                                         scale=-1.0, bias=1.0)
                else:
                    nc.vector.tensor_copy(out=xs, in_=xj)
                    nc.vector.tensor_scalar(out=qs, in0=xj, scalar1=-1.0,
                                            scalar2=1.0, op0=MUL, op1=ADD)
                nc.vector.tensor_tensor(out=x2s, in0=xs, in1=xs, op=MUL)
                nc.vector.tensor_tensor(out=q2s, in0=qs, in1=qs, op=MUL)
                # cubic terms written straight into Bbig (a=3: x^3, a=0: q^3)
                nc.vector.tensor_tensor(
                    out=_ap(Bbig[:, :, :, :], 3 * S + sl, db),
                    in0=x2s, in1=xs, op=MUL)
                nc.vector.tensor_tensor(
                    out=_ap(Bbig[:, :, :, :], 0 * S + sl, db),
                    in0=q2s, in1=qs, op=MUL)
                nc.vector.scalar_tensor_tensor(
                    out=_ap(Bbig[:, :, :, :], 1 * S + sl, db),
                    in0=xs, scalar=3.0, in1=q2s, op0=MUL, op1=MUL)
                nc.vector.scalar_tensor_tensor(
                    out=_ap(Bbig[:, :, :, :], 2 * S + sl, db),
                    in0=x2s, scalar=3.0, in1=qs, op0=MUL, op1=MUL)

            # ---- power chains on Pool: px/pq [p, s, d, m16] ----
            px = sg.tile([P, S, DIM, 16], F16)
            pq = sg.tile([P, S, DIM, 16], F16)
            SD = S * DIM

            def emit_chain(tbl, lvl1):
                t = tbl[:, :, :, :]
                nc.gpsimd.memset(_ap(t, 0, [[16, SD]]), 1.0)
                nc.gpsimd.tensor_copy(out=_ap(t, 1, [[16, SD]]),
                                      in_=lvl1[:, :, :])
                nc.gpsimd.tensor_tensor(
                    out=_ap(t, 2, [[16, SD]]), in0=_ap(t, 1, [[16, SD]]),
                    in1=_ap(t, 1, [[16, SD]]), op=MUL)
                nc.gpsimd.tensor_tensor(
                    out=_ap(t, 3, [[16, SD], [1, 2]]),
                    in0=_ap(t, 1, [[16, SD], [1, 2]]),
                    in1=_ap(t, 2, [[16, SD], [0, 2]]), op=MUL)
                nc.gpsimd.tensor_tensor(
                    out=_ap(t, 5, [[16, SD], [1, 4]]),
                    in0=_ap(t, 1, [[16, SD], [1, 4]]),
                    in1=_ap(t, 4, [[16, SD], [0, 4]]), op=MUL)
                nc.gpsimd.tensor_tensor(
                    out=_ap(t, 9, [[16, SD], [1, 7]]),
                    in0=_ap(t, 1, [[16, SD], [1, 7]]),
                    in1=_ap(t, 8, [[16, SD], [0, 7]]), op=MUL)

            # ---- Kron chain: k2, k3 ----
            k2 = sg.tile([P, 16, S], F16)      # (a0,a1) major, s inner
            k3 = sg.tile([P, 64, S], F16)      # (a0a1a2) major, s inner

            def emit_k2_range(sl, w):
                nc.vector.tensor_tensor(
                    out=_ap(k2[:, :, :], sl, [[4 * S, 4], [S, 4], [1, w]]),
                    in0=_ap(Bbig[:, :, :, :], sl, [[S, 4], [0, 4], [1, w]]),
                    in1=_ap(Bbig[:, :, :, :], 4 * S + sl,
                            [[0, 4], [S, 4], [1, w]]),
                    op=MUL)

            def emit_k3_range(sl, w):
                nc.vector.tensor_tensor(
                    out=_ap(k3[:, :, :], sl, [[4 * S, 16], [S, 4], [1, w]]),
                    in0=_ap(k2[:, :, :], sl, [[S, 16], [0, 4], [1, w]]),
                    in1=_ap(Bbig[:, :, :, :], 2 * 4 * S + sl,
                            [[0, 16], [S, 4], [1, w]]),
                    op=MUL)

            if FAST_START:
                emit_prep(0, G, cast_act=FASTCAST_ACT)
                emit_k2_range(0, G)
                emit_k3_range(0, G)
            else:
                emit_prep(0, S)
                emit_k2_range(0, S)
                emit_k3_range(0, 16)

            # ---- per-slot outputs ----
            phih = sg.tile([P, S, 96], F16)    # px*pq_rev for late slots
            dks = sg.tile([P, S, 96], F16)     # dk bands: cols 16i+m, i=0..5
            s6 = sg.tile([P, S, DIM], F16)
            e2 = sg.tile([P, S, 96], F16)
            dq = sg.tile([P, S], F32)

            emit_chain(px, xh6)
            emit_chain(pq, qh6)

            cg_tiles = {}

            def emit_c4(g):
                s0 = g * G
                cg = cgp.tile([P, 256, G], F16, tag=f"cg{g % CG_BUFS}")
                eng = nc.vector if C4_ENG[g] == "v" else nc.gpsimd
                eng.tensor_tensor(
                    out=cg[:, :, :],
                    in0=_ap(k3[:, :, :], s0, [[S, 64], [0, 4], [1, G]]),
                    in1=_ap(Bbig[:, :, :, :], 3 * 4 * S + s0,
                            [[0, 64], [S, 4], [1, G]]),
                    op=MUL)
                cg_tiles[g] = cg

            for g in range(NG):
                s0 = g * G
                if C4_EARLY:
                    k3_sched = {max(1, 16 // G - 2): (16, 16)}
                else:
                    k3_sched = {max(1, 16 // G - 2): (16, 16),
                                max(2, 32 // G - 2): (32, 16),
                                max(3, 48 // G - 2): (48, 16)}
                if g in k3_sched:
                    emit_k3_range(*k3_sched[g])
                # cond4 for this group: [p, c256, ds8]
                # first/last groups on DVE (pipeline start + tail), rest Pool
                ptr = ptrp.tile([P, 2 * G, P], F16, tag="ptr")

                def emit_transposes(cg, dl, dh):
                    for ds in range(dl, dh):
                        for h in range(2):
                            nc.tensor.matmul(
                                out=ptr[:, ds * 2 + h, :],
                                lhsT=_ap(cg[:, :, :], h * P * G + ds, [[G, P]]),
                                rhs=idh[:, :],
                                is_transpose=True, start=True, stop=True)

                if g == 0 and FAST_START and G0_SPLIT:
                    # split build+transpose halves for minimal lead-in
                    cg = cgp.tile([P, 256, G], F16, tag="cg0")
                    cg_tiles[0] = cg
                    NSP = G0_PIECES
                    GH = G // NSP
                    for hf in range(NSP):
                        nc.vector.tensor_tensor(
                            out=_ap(cg[:, :, :], hf * GH, [[G, 256], [1, GH]]),
                            in0=_ap(k3[:, :, :], hf * GH,
                                    [[S, 64], [0, 4], [1, GH]]),
                            in1=_ap(Bbig[:, :, :, :], 3 * 4 * S + hf * GH,
                                    [[0, 64], [S, 4], [1, GH]]),
                            op=MUL)
                        emit_transposes(cg, hf * GH, hf * GH + GH)
                else:
                    if g not in cg_tiles:
                        emit_c4(g)
                    cg = cg_tiles[g]
                    emit_transposes(cg, 0, G)

                # ct4 drain psum -> sbuf, distributed across engines.
                # lowercase = whole drain; split modes use two engines/halves.
                ct4 = ct4p.tile([P, G, 2, P], F16, tag="ct4")
                mode = CT4_DRAIN[g]
                if g == 0 and FAST_START and G0_SPLIT:
                    GH2 = 2 * G // G0_PIECES
                    halves = [("a", i * GH2, GH2) for i in range(G0_PIECES)]
                else:
                    G2 = 2 * G
                    halves = {"a": [("a", 0, G2)], "v": [("v", 0, G2)],
                              "p": [("p", 0, G2)],
                              "s": [("a", 0, G), ("v", G, G)],
                              "t": [("a", 0, G), ("p", G, G)],
                              "u": [("v", 0, G), ("p", G, G)]}[mode]
                for engc, off, w in halves:
                    de = {"a": nc.scalar, "v": nc.vector, "p": nc.gpsimd}[engc]
                    oap = _ap(ct4[:, :, :, :], off * P, [[1, w * P]])
                    iap = _ap(ptr[:, :, :], off * P, [[1, w * P]])
                    if de is nc.scalar:
                        de.copy(out=oap, in_=iap)
                    else:
                        de.tensor_copy(out=oap, in_=iap)

                # matmuls per slot: dims0-4 -> pdk, dim5 G5 -> pg5
                pdk = pdkp.tile([P, G, 128], F32, tag="pdk")
                pg5 = pg5p.tile([P, G, 64], F32, tag="pg5")
                for ds in range(G):
                    for h in range(2):
                        nc.tensor.matmul(
                            out=_ap(pdk[:, :, :], ds * 128, [[1, 80]]),
                            lhsT=ct4[:, ds, h, :], rhs=wcomb[:, h, :],
                            start=(h == 0), stop=(h == 1))
                    for h in range(2):
                        nc.tensor.matmul(
                            out=pg5[:, ds, :],
                            lhsT=ct4[:, ds, h, :], rhs=ca5w[:, h, :],
                            start=(h == 0), stop=(h == 1))

                # drain dims0-4 bands (fp32 psum -> fp16 sbuf)
                pe_ = {"a": nc.scalar, "v": nc.vector, "p": nc.gpsimd}[PDK_DRAIN[g]]
                if pe_ is nc.scalar:
                    pe_.copy(out=_ap(dks[:, :, :], s0 * 96, [[96, G], [1, 80]]),
                             in_=_ap(pdk[:, :, :], 0, [[128, G], [1, 80]]))
                else:
                    pe_.tensor_copy(
                        out=_ap(dks[:, :, :], s0 * 96, [[96, G], [1, 80]]),
                        in_=_ap(pdk[:, :, :], 0, [[128, G], [1, 80]]))

                # G5 combine: t5 = pg5 * B4, tree-sum over l.
                # GPSIMD cannot touch PSUM on HW: Act stages pg5 to SBUF first.
                t5 = scr.tile([P, G, 4, 16], F16, tag="t5")
                if T5_ENG[g] == "v":
                    t5_in0 = pg5[:, :, :]
                    t5e = nc.vector
                else:
                    pg5s = scr.tile([P, G, 64], F16, tag="pg5s")
                    nc.scalar.copy(out=pg5s[:, :, :], in_=pg5[:, :, :])
                    t5_in0 = pg5s[:, :, :]
                    t5e = nc.gpsimd
                t5e.tensor_tensor(
                    out=t5[:, :, :, :],
                    in0=t5_in0,
                    in1=_ap(Bbig[:, :, :, :], 4 * 4 * S + s0,
                            [[1, G], [S, 4], [0, 16]]),
                    op=MUL)
                u5 = scr.tile([P, G, 2, 16], F16, tag="u5")
                u5e = nc.vector if U5D5_ENG[g] == "v" else nc.gpsimd
                u5e.tensor_tensor(
                    out=u5[:, :, :, :],
                    in0=_ap(t5[:, :, :, :], 0, [[64, G], [1, 32]]),
                    in1=_ap(t5[:, :, :, :], 32, [[64, G], [1, 32]]), op=ADD)
                u5e.tensor_tensor(
                    out=_ap(dks[:, :, :], s0 * 96 + 80, [[96, G], [1, 16]]),
                    in0=_ap(u5[:, :, :, :], 0, [[32, G], [1, 16]]),
                    in1=_ap(u5[:, :, :, :], 16, [[32, G], [1, 16]]), op=ADD)

                # combine: e1 = dk*px, e2 = e1*pq_rev
                for ci, (sb, se) in enumerate(COMBINE_SCHED.get(g, ())):
                    cidx = sum(len(v) for k, v in COMBINE_SCHED.items()
                               if k < g) + ci
                    me = nc.vector if M1M2_ENG[cidx] == "v" else nc.gpsimd
                    W2 = se - sb
                    if sb >= PHIH_FROM:
                        me.tensor_tensor(
                            out=_ap(e2[:, :, :], sb * 96, [[96, W2], [1, 96]]),
                            in0=_ap(dks[:, :, :], sb * 96, [[96, W2], [1, 96]]),
                            in1=_ap(phih[:, :, :], sb * 96, [[96, W2], [1, 96]]),
                            op=MUL)
                        continue
                    e1 = scr.tile([P, 16, 96], F16, tag="e1")
                    me.tensor_tensor(
                        out=_ap(e1[:, :, :], 0, [[96, W2], [1, 96]]),
                        in0=_ap(dks[:, :, :], sb * 96, [[96, W2], [1, 96]]),
                        in1=_ap(px[:, :, :, :], sb * 96,
                                [[96, W2], [16, 6], [1, 16]]),
                        op=MUL)
                    me.tensor_tensor(
                        out=_ap(e2[:, :, :], sb * 96, [[96, W2], [1, 96]]),
                        in0=_ap(e1[:, :, :], 0, [[96, W2], [1, 96]]),
                        in1=_ap(pq[:, :, :, :], sb * 96 + 15,
                                [[96, W2], [16, 6], [-1, 16]]),
                        op=MUL)
                # tree-reduce over m + dim product
                for ti, (sb, se) in enumerate(TREE_SCHED.get(g, ())):
                    tidx = sum(len(v) for k, v in TREE_SCHED.items()
                               if k < g) + ti
                    te_ = nc.vector if TREE_ENG[tidx] == "v" else nc.gpsimd
                    H = se - sb
                    ta = scr.tile([P, 32, 6, 8], F16, tag="ta")
                    te_.tensor_tensor(
                        out=_ap(ta[:, :, :, :], 0, [[48, H], [8, 6], [1, 8]]),
                        in0=_ap(e2[:, :, :], sb * 96, [[96, H], [16, 6], [1, 8]]),
                        in1=_ap(e2[:, :, :], sb * 96 + 8,
                                [[96, H], [16, 6], [1, 8]]), op=ADD)
                    tb = scr.tile([P, 32, 6, 4], F16, tag="tb")
                    te_.tensor_tensor(
                        out=_ap(tb[:, :, :, :], 0, [[24, H], [4, 6], [1, 4]]),
                        in0=_ap(ta[:, :, :, :], 0, [[48, H], [8, 6], [1, 4]]),
                        in1=_ap(ta[:, :, :, :], 4, [[48, H], [8, 6], [1, 4]]),
                        op=ADD)
                    tc = scr.tile([P, 32, 6, 2], F16, tag="tc")
                    te_.tensor_tensor(
                        out=_ap(tc[:, :, :, :], 0, [[12, H], [2, 6], [1, 2]]),
                        in0=_ap(tb[:, :, :, :], 0, [[24, H], [4, 6], [1, 2]]),
                        in1=_ap(tb[:, :, :, :], 2, [[24, H], [4, 6], [1, 2]]),
                        op=ADD)
                    te_.tensor_tensor(
                        out=_ap(s6[:, :, :], sb * DIM, [[DIM, H], [1, DIM]]),
                        in0=_ap(tc[:, :, :, :], 0, [[12, H], [2, DIM]]),
                        in1=_ap(tc[:, :, :, :], 1, [[12, H], [2, DIM]]), op=ADD)
                    # final product over dims for this s-range
                    r1 = scr.tile([P, 32, 3], F16, tag="r1")
                    te_.tensor_tensor(
                        out=_ap(r1[:, :, :], 0, [[3, H], [1, 3]]),
                        in0=_ap(s6[:, :, :], sb * DIM, [[DIM, H], [2, 3]]),
                        in1=_ap(s6[:, :, :], sb * DIM + 1, [[DIM, H], [2, 3]]),
                        op=MUL)
                    r2 = scr.tile([P, 32], F16, tag="r2")
                    te_.tensor_tensor(
                        out=_ap(r2[:, :], 0, [[1, H]]),
                        in0=_ap(r1[:, :, :], 0, [[3, H]]),
                        in1=_ap(r1[:, :, :], 1, [[3, H]]), op=MUL)
                    te_.tensor_tensor(
                        out=_ap(dq[:, :], sb, [[1, H]]),
                        in0=_ap(r2[:, :], 0, [[1, H]]),
                        in1=_ap(r1[:, :, :], 2, [[3, H]]), op=MUL)
                    if DENS_SPLIT:
                        nc.sync.dma_start(
                            out=_ap(dens_out[:, :], sb, [[1, H]]),
                            in_=_ap(dq[:, :], sb, [[1, H]]))

                if FAST_START and g == 0:
                    # remaining slots' prep, overlapped with group-0 pipeline
                    emit_prep(G, S - G, cast_act=True)
                    emit_k2_range(G, S - G)
                    emit_k3_range(G, 16 - G)
                phih_sched = {3: (16, 32), 4: (32, 48), 5: (48, 64)}
                if g in phih_sched and phih_sched[g][0] >= PHIH_FROM:
                    pb, pe2 = phih_sched[g]
                    Wp = pe2 - pb
                    nc.gpsimd.tensor_tensor(
                        out=_ap(phih[:, :, :], pb * 96, [[96, Wp], [1, 96]]),
                        in0=_ap(px[:, :, :, :], pb * 96,
                                [[96, Wp], [16, 6], [1, 16]]),
                        in1=_ap(pq[:, :, :, :], pb * 96 + 15,
                                [[96, Wp], [16, 6], [-1, 16]]),
                        op=MUL)
                if C4_EARLY and g == 1:
                    # build late DVE groups' cond4 during the early stall
                    emit_k3_range(32, 32)
                    for gl in range(NG - 16 // G, NG):
                        if C4_ENG[gl] == "v":
                            emit_c4(gl)

            if not DENS_SPLIT:
                nc.sync.dma_start(out=dens_out[:, :], in_=dq[:, :])

    nc.finalize()
    return nc


def _softplus64(v):
    return np.logaddexp(0.0, v)


def _host_consts(As):
    """wcomb [128,2,80] f16 and ca5w [128,2,64] f16 from fp64 W'' matrices."""
    kap = 16.0 * np.array([math.comb(15, m) for m in range(16)], dtype=np.float64)
    W = []
    for i in range(DIM):
        c = np.cumsum(_softplus64(As[i].astype(np.float64)), axis=1)
        ca = 2.0 * (1.0 / (1.0 + np.exp(-c)) - 0.5)
        ca_ext = np.concatenate(
            [np.zeros((ca.shape[0], 1)), ca, np.ones((ca.shape[0], 1))], axis=1)
        W.append(kap[None, :] * (ca_ext[:, 1:] - ca_ext[:, :-1]))  # [rows,16]

    wcomb = np.zeros((P, 2, 80), dtype=np.float64)
    for h in range(2):
        for p in range(P):
            c4 = 128 * h + p
            wcomb[p, h, 0:16] = W[0][0]
            wcomb[p, h, 16:32] = W[1][c4 >> 6]
            wcomb[p, h, 32:48] = W[2][c4 >> 4]
            wcomb[p, h, 48:64] = W[3][c4 >> 2]
            wcomb[p, h, 64:80] = W[4][c4]
    ca5w = np.zeros((P, 2, 64), dtype=np.float64)
    for h in range(2):
        for p in range(P):
            for l in range(4):
                ca5w[p, h, l * 16:(l + 1) * 16] = W[5][(128 * h + p) * 4 + l]
    return wcomb.astype(np.float16), ca5w.astype(np.float16)


def kernel(**inputs):
    x = np.asarray(inputs["x"], dtype=np.float32)
    As = [np.asarray(inputs[f"A{i}"], dtype=np.float32) for i in range(DIM)]

    if "nc" not in _CACHE:
        _CACHE["nc"] = _build_nc()
    nc = _CACHE["nc"]

    wcomb, ca5w = _host_consts(As)
    idh = np.eye(P, dtype=np.float16)

    in_maps = []
    for c in range(NCORES):
        xc = x[c * NC:(c + 1) * NC].reshape(P, S, DIM)
        in_maps.append({"xr": xc, "wcomb": wcomb, "ca5w": ca5w, "idh": idh})

    res = run_bass_kernel_spmd(nc, in_maps, core_ids=list(range(NCORES)))
    outs = [r["dens"].reshape(NC) for r in res.results]
    return np.concatenate(outs, axis=0)


if __name__ == "__main__":
    rng = np.random.default_rng(0)
    ins = {"x": rng.uniform(0, 1, (N, DIM)).astype(np.float32)}
    for i in range(DIM):
        ins[f"A{i}"] = rng.uniform(0, 1, ((4 ** i), 15)).astype(np.float32)
    out = kernel(**ins)
    print(out.shape, out[:4])
